# revision 16
# baseline (speedup 1.0000x reference)
"""Trainium2 Bass kernel for nn_Head (sparse attention head), v2.

Computation (per batch b):
    K = X @ Wk; Q = X @ Wq; V = X @ Wv                       # [T, HS]
    S = Q K^T / sqrt(HS)                                     # [T, T]
    A = softmax_row(where(dag==0, -inf, S))                  # row-wise over keys
    out[j, h] = sum_i A[i, j] V[i, h]   (transposed AV)      # [T, HS]
    return swish(out)

Sharding over 8 NeuronCores: core = (b, h) with b = batch (4), h = query-row
half (2); each core handles TH=2048 query rows against all T=4096 keys.
The host rolls X^T columns (and mask key columns identically) so the shard's
query rows always sit in xt columns [0, TH) -- one program serves both h=0
and h=1 cores; the host unrolls the outputs.

v2 structure (vs v1 baseline):
  - mask shipped as fp8 keep-mask {0,1} (half the DMA bytes of f16)
  - 1/4 of score quarters get the mask applied ON THE TENSOR ENGINE: a
    diag(240) fp8 stationary matmul adds 240*keep into the score PSUM and
    the activation uses bias=-30 so exp(0.125*s + 30*keep - 30) kills
    masked entries (exp(<= -24) flushes to 0 in f16); activation accum_out
    then yields the masked row-sum for free on the Scalar engine.
  - remaining quarters keep v1's DVE scalar_tensor_tensor (exp * mask with
    accumulate) so Tensor/Vector/Scalar all carry ~equal load.
  - the transposed-AV matmul accumulates into a partition-packed PSUM tile
    [128, 2048] (output column halves stacked along partitions); block k's
    AV matmuls are interleaved into block k+1's score quarters so the PE
    never idles (keeps the HAM clock-gate at 2.4 GHz) and phase C vanishes.
  - matmuls are grouped by stationary operand to minimize LDWEIGHTS churn;
    phase A runs contraction-chunk-major with one weight load per chunk.
"""

import sys

for _p in ("/opt/trn_rl_repo",):
    if _p not in sys.path:
        sys.path.append(_p)

import numpy as np
import ml_dtypes

import concourse.bacc as bacc
import concourse.mybir as mybir
import concourse.tile as tile
from concourse.bass_utils import run_bass_kernel_spmd

B, T, D, HS = 4, 4096, 512, 64
TH = T // 2          # query rows per core
P = 128              # partitions
NB = TH // P         # 16 query blocks per core
NCC = D // P         # 4 contraction chunks over D
NJ = 512             # phase-A matmul free dim / PSUM bank width
NQ = 1024            # phase-B quarter width (keys)
VSCALE = 1024.0      # dynamic-range scale folded into V/l

F8 = mybir.dt.float8e4
F16 = mybir.dt.float16
F32 = mybir.dt.float32
AF = mybir.ActivationFunctionType
ALU = mybir.AluOpType
AX = mybir.AxisListType

NP_F8 = ml_dtypes.float8_e4m3


def _inject(k, jq):
    # score quarters whose mask goes in via tensor-engine injection
    # (the rest use the DVE scalar_tensor_tensor path)
    return jq in (0, 3)


_CACHE = {}


def _build():
    if "nc" in _CACHE:
        return _CACHE["nc"]

    nc = bacc.Bacc("TRN2", target_bir_lowering=False, debug=False)

    # register the -30.0 activation-bias constant (only 0.0/1.0 exist by default)
    _bias_t = nc.alloc_sbuf_tensor("const-float32--30.0", [128, 1], F32)
    nc.gpsimd.memset(_bias_t.ap(), -30.0)
    nc.const_aps.aps[(F32, -30.0)] = _bias_t.ap()
    nc.all_engine_barrier()

    xt_d = nc.dram_tensor("xt", [D, T], F16, kind="ExternalInput").ap()
    mk_d = nc.dram_tensor("mk", [TH, T], F8, kind="ExternalInput").ap()
    wkq_d = nc.dram_tensor("wkq", [D, P], F16, kind="ExternalInput").ap()
    wv_d = nc.dram_tensor("wv", [D, HS], F16, kind="ExternalInput").ap()
    idn_d = nc.dram_tensor("idn", [P, P], F8, kind="ExternalInput").ap()
    idh_d = nc.dram_tensor("idh", [64, 64], F16, kind="ExternalInput").ap()
    ot_d = nc.dram_tensor("ot", [P, TH], F32, kind="ExternalOutput").ap()

    with tile.TileContext(nc) as tc:
        with tc.tile_pool(name="persist", bufs=1) as pp:
            kt = pp.tile([64, T], F16, tag="kt")         # K^T
            qt = pp.tile([64, TH], F16, tag="qt")        # Q^T (shard rows)
            v = pp.tile([P, NB * HS], F16, tag="v")      # V rows (shard)
            vt = pp.tile([P, NB * HS], F16, tag="vt")    # V/l * VSCALE
            idn = pp.tile([P, P], F8, tag="idn")         # diag(240)
            idh = pp.tile([64, 64], F16, tag="idh")      # f16 identity
            nc.sync.dma_start(idn[:], idn_d[:, :])
            nc.sync.dma_start(idh[:], idh_d[:, :])

            # PE warmup: dummy matmuls keep the PE busy through the HAM
            # activity window so the clock un-throttles to 2.4 GHz while
            # the X^T DMA streams in; they depend only on the tiny idn DMA.
            with tc.tile_pool(name="psW", bufs=1, space="PSUM") as psW:
                wsp = psW.tile([P, P], F32, tag="wsp")
                for _ in range(56):
                    nc.tensor.matmul(wsp[:], idn[:], idn[:], start=True, stop=True)

            # mask pool opens early: the first blocks' masks load before
            # the big X^T transfers so phase B can start the moment the
            # projections finish
            pM_cm = tc.tile_pool(name="pM", bufs=4)
            pM = pM_cm.__enter__()
            mk_pre = []
            for k in range(2):
                for jh in range(2):
                    mkp = pM.tile([P, 2 * NQ], F8, tag="mk", name=f"mkp{k}{jh}")
                    nc.sync.dma_start(
                        mkp[:],
                        mk_d[k * P:(k + 1) * P, jh * 2 * NQ:(jh + 1) * 2 * NQ],
                    )
                    mk_pre.append(mkp)

            # ---- phase A: load X^T / weights, compute K^T, Q^T, V ----
            # All projection outputs are partition-PACKED [128, *] (two 64-row
            # halves stacked) so every PSUM bank carries real work and the
            # whole phase runs chunk-major, riding the X^T DMA ladder.
            with tc.tile_pool(name="phA", bufs=1) as pA:
                xt = pA.tile([P, NCC * T], F16, tag="xt")
                wkq = pA.tile([P, NCC * P], F16, tag="wkq")
                wv = pA.tile([P, NCC * HS], F16, tag="wv")
                vtt = pA.tile([64, TH], F16, tag="vtt")  # V^T (pre-transpose)
                for ci in range(NCC):
                    cs = slice(ci * P, (ci + 1) * P)
                    nc.sync.dma_start(wkq[:, ci * P:(ci + 1) * P], wkq_d[cs, :])
                    nc.sync.dma_start(wv[:, ci * HS:(ci + 1) * HS], wv_d[cs, :])
                # fine-grained chunk-major DMA: 64 transfers ladder across the
                # 16 queues so chunk ci is resident ~3us after chunk ci-1
                NDB = 512
                for half in range(2):
                    for ci in range(NCC):
                        cs = slice(ci * P, (ci + 1) * P)
                        for j0 in range(half * TH, half * TH + TH, NDB):
                            nc.sync.dma_start(
                                xt[:, ci * T + j0: ci * T + j0 + NDB],
                                xt_d[cs, j0:j0 + NDB],
                            )

                with tc.tile_pool(name="psA", bufs=8, space="PSUM") as psA:
                    # rotating 8-bank pool: K^T tiles flow out as Q^T/V^T
                    # tiles flow in, so projections pipeline with the copies
                    NKH = TH // NJ
                    ktp = [psA.tile([64, NJ], F32, tag="pa", name=f"ktp{j}")
                           for j in range(NKH)]
                    qtp = [psA.tile([64, NJ], F32, tag="pa", name=f"qtp{j}")
                           for j in range(NKH)]
                    for ci in range(NCC):
                        for j0 in range(NKH):
                            nc.tensor.matmul(
                                ktp[j0][:],
                                wkq[:, ci * P: ci * P + 64],
                                xt[:, ci * T + j0 * NJ: ci * T + (j0 + 1) * NJ],
                                start=(ci == 0),
                                stop=(ci == NCC - 1),
                            )
                        for j0 in range(NKH):
                            nc.tensor.matmul(
                                qtp[j0][:],
                                wkq[:, ci * P + 64:(ci + 1) * P],
                                xt[:, ci * T + j0 * NJ: ci * T + (j0 + 1) * NJ],
                                start=(ci == 0),
                                stop=(ci == NCC - 1),
                            )
                    for j0 in range(NKH):
                        nc.vector.tensor_copy(
                            kt[:, j0 * NJ:(j0 + 1) * NJ], ktp[j0][:]
                        )
                        nc.vector.tensor_copy(
                            qt[:, j0 * NJ:(j0 + 1) * NJ], qtp[j0][:]
                        )
                    vtp = [psA.tile([64, NJ], F32, tag="pa", name=f"vtp{j}")
                           for j in range(NKH)]
                    kt2 = [psA.tile([64, NJ], F32, tag="pa", name=f"kt2{j}")
                           for j in range(NKH)]
                    for ci in range(NCC):
                        for j0 in range(NKH):
                            nc.tensor.matmul(
                                vtp[j0][:],
                                wv[:, ci * HS:(ci + 1) * HS],
                                xt[:, ci * T + j0 * NJ: ci * T + (j0 + 1) * NJ],
                                start=(ci == 0),
                                stop=(ci == NCC - 1),
                            )
                        for j0 in range(NKH):
                            nc.tensor.matmul(
                                kt2[j0][:],
                                wkq[:, ci * P: ci * P + 64],
                                xt[:, ci * T + TH + j0 * NJ:
                                   ci * T + TH + (j0 + 1) * NJ],
                                start=(ci == 0),
                                stop=(ci == NCC - 1),
                            )
                    for j0 in range(NKH):
                        nc.scalar.copy(
                            vtt[:, j0 * NJ:(j0 + 1) * NJ], vtp[j0][:]
                        )
                        nc.vector.tensor_copy(
                            kt[:, TH + j0 * NJ: TH + (j0 + 1) * NJ], kt2[j0][:]
                        )
                    # transposes ride the same rotating pool
                    for kg in range(4):
                        vtr = psA.tile([P, 4 * HS], F16, tag="pa", name=f"vtr{kg}")
                        for kk in range(4):
                            k = kg * 4 + kk
                            nc.tensor.transpose(
                                vtr[:, kk * HS:(kk + 1) * HS],
                                vtt[:, k * P:(k + 1) * P],
                                idh[:],
                            )
                        nc.vector.tensor_copy(
                            v[:, kg * 4 * HS:(kg + 1) * 4 * HS], vtr[:]
                        )

            # ---- phase B: scores, masked exp, row-sums, interleaved AV ----
            with (
                tc.tile_pool(name="psB", bufs=2, space="PSUM") as psB,
                tc.tile_pool(name="psC", bufs=1, space="PSUM") as psC,
                tc.tile_pool(name="pU", bufs=2) as pU,
                tc.tile_pool(name="pE", bufs=3) as pE,
                tc.tile_pool(name="pL", bufs=2) as pL,
            ):
                otp = psC.tile([P, TH], F32, tag="otp")  # packed AV accumulator

                def av_matmuls(k, u, jh):
                    # 4 of block k's AV matmuls: output partition half jh
                    for j2 in range(4):
                        nc.tensor.matmul(
                            otp[jh * 64:(jh + 1) * 64, j2 * NJ:(j2 + 1) * NJ],
                            vt[:, k * HS:(k + 1) * HS],
                            u[:, jh * TH + j2 * NJ: jh * TH + (j2 + 1) * NJ],
                            start=(k == 0),
                            stop=(k == NB - 1),
                            skip_group_check=True,
                        )

                def warm_fill(n):
                    # junk matmuls to keep the PE streaming where no AV work
                    # exists yet (start of phase B); output is overwritten by
                    # the next real start=True matmul on that buffer
                    wj = psB.tile([P, NQ], F32, tag="sp", name="wjunk")
                    for _ in range(n):
                        nc.tensor.matmul(
                            wj[:, 0:P], idn[:], idn[:], start=True, stop=True
                        )

                def quarter(k, jq, mk, jqh, u, ll):
                    sp = psB.tile([P, NQ], F32, tag="sp", name="sp")
                    inj = _inject(k, jq)
                    if inj:
                        for jb in range(2):
                            nc.tensor.matmul(
                                sp[:, jb * NJ:(jb + 1) * NJ],
                                idn[:],
                                mk[:, jqh * NQ + jb * NJ:
                                   jqh * NQ + (jb + 1) * NJ],
                                start=True,
                                stop=False,
                            )
                    for jb in range(2):
                        nc.tensor.matmul(
                            sp[:, jb * NJ:(jb + 1) * NJ],
                            qt[:, k * P:(k + 1) * P],
                            kt[:, jq * NQ + jb * NJ: jq * NQ + (jb + 1) * NJ],
                            start=(not inj),
                            stop=True,
                        )
                    if inj:
                        nc.scalar.activation(
                            u[:, jq * NQ:(jq + 1) * NQ], sp[:], AF.Exp,
                            bias=-30.0, scale=0.125,
                            accum_out=ll[:, jq:jq + 1],
                        )
                    else:
                        er = pE.tile([P, NQ], F16, tag="er", name="er")
                        nc.scalar.activation(er[:], sp[:], AF.Exp, scale=0.125)
                        nc.vector.scalar_tensor_tensor(
                            out=u[:, jq * NQ:(jq + 1) * NQ],
                            in0=er[:],
                            scalar=1.0,
                            in1=mk[:, jqh * NQ:(jqh + 1) * NQ],
                            op0=ALU.mult,
                            op1=ALU.mult,
                            accum_out=ll[:, jq:jq + 1],
                        )

                prev = None  # (k, u) whose AV matmuls are still pending
                for k in range(NB):
                    u = pU.tile([P, T], F16, tag="u")
                    ll = pL.tile([P, 4], F32, tag="ll")
                    if k < 2:
                        mk0, mk1 = mk_pre[2 * k], mk_pre[2 * k + 1]
                    else:
                        mk0 = pM.tile([P, 2 * NQ], F8, tag="mk", name="mk0")
                        nc.sync.dma_start(
                            mk0[:], mk_d[k * P:(k + 1) * P, 0:2 * NQ]
                        )
                        mk1 = pM.tile([P, 2 * NQ], F8, tag="mk", name="mk1")
                        nc.sync.dma_start(
                            mk1[:], mk_d[k * P:(k + 1) * P, 2 * NQ:4 * NQ]
                        )
                    quarter(k, 0, mk0, 0, u, ll)
                    quarter(k, 1, mk0, 1, u, ll)
                    # previous block's AV matmuls slot here: their inputs are
                    # long since ready, so they keep the PE streaming while
                    # exp works through this block's quarters
                    if prev is not None:
                        av_matmuls(prev[0], prev[1], 0)
                    else:
                        warm_fill(8)
                    quarter(k, 2, mk1, 0, u, ll)
                    if prev is not None:
                        av_matmuls(prev[0], prev[1], 1)
                    else:
                        warm_fill(8)
                    quarter(k, 3, mk1, 1, u, ll)
                    lt = pL.tile([P, 1], F32, tag="lt")
                    nc.vector.tensor_reduce(lt[:], ll[:], AX.X, ALU.add)
                    rl = pL.tile([P, 1], F32, tag="rl")
                    nc.vector.reciprocal(rl[:], lt[:])
                    nc.vector.tensor_scalar(
                        out=vt[:, k * HS:(k + 1) * HS],
                        in0=v[:, k * HS:(k + 1) * HS],
                        scalar1=rl[:],
                        scalar2=VSCALE,
                        op0=ALU.mult,
                        op1=ALU.mult,
                    )
                    prev = (k, u)
                av_matmuls(prev[0], prev[1], 0)
                av_matmuls(prev[0], prev[1], 1)

                with tc.tile_pool(name="phO", bufs=1) as pO:
                    ot_sb = pO.tile([P, TH], F32, tag="ot_sb")
                    for hh in range(2):
                        cs = slice(hh * (TH // 2), (hh + 1) * (TH // 2))
                        nc.vector.tensor_copy(ot_sb[:, cs], otp[:, cs])
                        nc.sync.dma_start(ot_d[:, cs], ot_sb[:, cs])
            pM_cm.__exit__(None, None, None)

    nc.compile()
    _CACHE["nc"] = nc
    return nc


def _prep_inputs(X, dag, Wk, Wq, Wv):
    X = np.asarray(X, dtype=np.float32)
    dag = np.asarray(dag)
    wkq = np.concatenate(
        [np.asarray(Wk, dtype=np.float16), np.asarray(Wq, dtype=np.float16)], axis=1
    )
    wv16 = np.asarray(Wv, dtype=np.float16)
    keep8 = (dag != 0).astype(NP_F8)  # [T, T] keep mask
    idn = np.zeros((P, P), dtype=NP_F8)
    np.fill_diagonal(idn, NP_F8(240.0))
    idh = np.eye(64, dtype=np.float16)
    in_maps = []
    for core in range(8):
        b, h = divmod(core, 2)
        xb = X[b].astype(np.float16)
        if h == 0:
            xt = np.ascontiguousarray(xb.T)
            mk = np.ascontiguousarray(keep8[0:TH])
        else:
            xt = np.ascontiguousarray(np.roll(xb.T, -TH, axis=1))
            mk = np.ascontiguousarray(np.roll(keep8[TH:2 * TH], -TH, axis=1))
        in_maps.append(
            {"xt": xt, "mk": mk, "wkq": wkq, "wv": wv16, "idn": idn, "idh": idh}
        )
    return in_maps


def kernel(X, dag, Wk, Wq, Wv, _trace=False):
    nc = _build()
    in_maps = _prep_inputs(X, dag, Wk, Wq, Wv)
    res = run_bass_kernel_spmd(nc, in_maps, list(range(8)), trace=_trace)
    out = np.empty((B, T, HS), dtype=np.float32)
    for b in range(B):
        acc = None
        for h in range(2):
            ot = res.results[2 * b + h]["ot"]  # [128, 2048] packed
            full = np.concatenate([ot[0:64, :], ot[64:128, :]], axis=1)  # [64, T]
            if h == 1:
                full = np.roll(full, TH, axis=1)  # undo key roll
            acc = full if acc is None else acc + full
        o = acc.T / np.float32(VSCALE)
        out[b] = o / (1.0 + np.exp(-o))  # swish
    if _trace:
        return out, res
    return out


# revision 17
# speedup vs baseline: 1.0291x; 1.0291x over previous
"""Trainium2 Bass kernel for nn_Head (sparse attention head), v2.

Computation (per batch b):
    K = X @ Wk; Q = X @ Wq; V = X @ Wv                       # [T, HS]
    S = Q K^T / sqrt(HS)                                     # [T, T]
    A = softmax_row(where(dag==0, -inf, S))                  # row-wise over keys
    out[j, h] = sum_i A[i, j] V[i, h]   (transposed AV)      # [T, HS]
    return swish(out)

Sharding over 8 NeuronCores: core = (b, h) with b = batch (4), h = query-row
half (2); each core handles TH=2048 query rows against all T=4096 keys.
The host rolls X^T columns (and mask key columns identically) so the shard's
query rows always sit in xt columns [0, TH) -- one program serves both h=0
and h=1 cores; the host unrolls the outputs.

v2 structure (vs v1 baseline):
  - mask shipped as fp8 keep-mask {0,1} (half the DMA bytes of f16)
  - 1/4 of score quarters get the mask applied ON THE TENSOR ENGINE: a
    diag(240) fp8 stationary matmul adds 240*keep into the score PSUM and
    the activation uses bias=-30 so exp(0.125*s + 30*keep - 30) kills
    masked entries (exp(<= -24) flushes to 0 in f16); activation accum_out
    then yields the masked row-sum for free on the Scalar engine.
  - remaining quarters keep v1's DVE scalar_tensor_tensor (exp * mask with
    accumulate) so Tensor/Vector/Scalar all carry ~equal load.
  - the transposed-AV matmul accumulates into a partition-packed PSUM tile
    [128, 2048] (output column halves stacked along partitions); block k's
    AV matmuls are interleaved into block k+1's score quarters so the PE
    never idles (keeps the HAM clock-gate at 2.4 GHz) and phase C vanishes.
  - matmuls are grouped by stationary operand to minimize LDWEIGHTS churn;
    phase A runs contraction-chunk-major with one weight load per chunk.
"""

import sys

for _p in ("/opt/trn_rl_repo",):
    if _p not in sys.path:
        sys.path.append(_p)

import numpy as np
import ml_dtypes

import concourse.bacc as bacc
import concourse.mybir as mybir
import concourse.tile as tile
from concourse.bass_utils import run_bass_kernel_spmd

B, T, D, HS = 4, 4096, 512, 64
TH = T // 2          # query rows per core
P = 128              # partitions
NB = TH // P         # 16 query blocks per core
NCC = D // P         # 4 contraction chunks over D
NJ = 512             # phase-A matmul free dim / PSUM bank width
NQ = 1024            # phase-B quarter width (keys)
VSCALE = 1024.0      # dynamic-range scale folded into V/l

F8 = mybir.dt.float8e4
F16 = mybir.dt.float16
F32 = mybir.dt.float32
AF = mybir.ActivationFunctionType
ALU = mybir.AluOpType
AX = mybir.AxisListType

NP_F8 = ml_dtypes.float8_e4m3


def _inject(k, jq):
    # score quarters whose mask goes in via tensor-engine injection
    # (the rest use the DVE scalar_tensor_tensor path)
    return jq in (0, 3)


_CACHE = {}


def _build():
    if "nc" in _CACHE:
        return _CACHE["nc"]

    nc = bacc.Bacc("TRN2", target_bir_lowering=False, debug=False)

    # register the -30.0 activation-bias constant (only 0.0/1.0 exist by default)
    _bias_t = nc.alloc_sbuf_tensor("const-float32--30.0", [128, 1], F32)
    nc.gpsimd.memset(_bias_t.ap(), -30.0)
    nc.const_aps.aps[(F32, -30.0)] = _bias_t.ap()
    nc.all_engine_barrier()

    xt_d = nc.dram_tensor("xt", [D, T], F16, kind="ExternalInput").ap()
    mk_d = nc.dram_tensor("mk", [TH, T], F8, kind="ExternalInput").ap()
    wkq_d = nc.dram_tensor("wkq", [D, P], F16, kind="ExternalInput").ap()
    wv_d = nc.dram_tensor("wv", [D, HS], F16, kind="ExternalInput").ap()
    idn_d = nc.dram_tensor("idn", [P, P], F8, kind="ExternalInput").ap()
    idh_d = nc.dram_tensor("idh", [64, 64], F16, kind="ExternalInput").ap()
    ot_d = nc.dram_tensor("ot", [P, TH], F32, kind="ExternalOutput").ap()

    with tile.TileContext(nc) as tc:
        with tc.tile_pool(name="persist", bufs=1) as pp:
            kt = pp.tile([64, T], F16, tag="kt")         # K^T
            qt = pp.tile([64, TH], F16, tag="qt")        # Q^T (shard rows)
            v = pp.tile([P, NB * HS], F16, tag="v")      # V rows (shard)
            vt = pp.tile([P, NB * HS], F16, tag="vt")    # V/l * VSCALE
            idn = pp.tile([P, P], F8, tag="idn")         # diag(240)
            idh = pp.tile([64, 64], F16, tag="idh")      # f16 identity
            nc.sync.dma_start(idn[:], idn_d[:, :])
            nc.sync.dma_start(idh[:], idh_d[:, :])

            # PE warmup: dummy matmuls keep the PE busy through the HAM
            # activity window so the clock un-throttles to 2.4 GHz while
            # the X^T DMA streams in; they depend only on the tiny idn DMA.
            with tc.tile_pool(name="psW", bufs=1, space="PSUM") as psW:
                wsp = psW.tile([P, P], F32, tag="wsp")
                for _ in range(56):
                    nc.tensor.matmul(wsp[:], idn[:], idn[:], start=True, stop=True)

            # mask pool opens early: the first blocks' masks load before
            # the big X^T transfers so phase B can start the moment the
            # projections finish
            pM_cm = tc.tile_pool(name="pM", bufs=4)
            pM = pM_cm.__enter__()
            mk_pre = []
            for k in range(2):
                for jh in range(2):
                    mkp = pM.tile([P, 2 * NQ], F8, tag="mk", name=f"mkp{k}{jh}")
                    nc.sync.dma_start(
                        mkp[:],
                        mk_d[k * P:(k + 1) * P, jh * 2 * NQ:(jh + 1) * 2 * NQ],
                    )
                    mk_pre.append(mkp)

            # ---- phase A: load X^T / weights, compute K^T, Q^T, V ----
            # All projection outputs are partition-PACKED [128, *] (two 64-row
            # halves stacked) so every PSUM bank carries real work and the
            # whole phase runs chunk-major, riding the X^T DMA ladder.
            with tc.tile_pool(name="phA", bufs=1) as pA:
                xt = pA.tile([P, NCC * T], F16, tag="xt")
                wkq = pA.tile([P, NCC * P], F16, tag="wkq")
                wv = pA.tile([P, NCC * HS], F16, tag="wv")
                vtt = pA.tile([64, TH], F16, tag="vtt")  # V^T (pre-transpose)
                for ci in range(NCC):
                    cs = slice(ci * P, (ci + 1) * P)
                    nc.sync.dma_start(wkq[:, ci * P:(ci + 1) * P], wkq_d[cs, :])
                    nc.sync.dma_start(wv[:, ci * HS:(ci + 1) * HS], wv_d[cs, :])
                # fine-grained chunk-major DMA: 64 transfers ladder across the
                # 16 queues so chunk ci is resident ~3us after chunk ci-1
                NDB = 1024
                for half in range(2):
                    for ci in range(NCC):
                        cs = slice(ci * P, (ci + 1) * P)
                        for j0 in range(half * TH, half * TH + TH, NDB):
                            nc.sync.dma_start(
                                xt[:, ci * T + j0: ci * T + j0 + NDB],
                                xt_d[cs, j0:j0 + NDB],
                            )

                with tc.tile_pool(name="psA", bufs=8, space="PSUM") as psA:
                    # rotating 8-bank pool: K^T tiles flow out as Q^T/V^T
                    # tiles flow in, so projections pipeline with the copies
                    NKH = TH // NJ
                    ktp = [psA.tile([64, NJ], F32, tag="pa", name=f"ktp{j}")
                           for j in range(NKH)]
                    qtp = [psA.tile([64, NJ], F32, tag="pa", name=f"qtp{j}")
                           for j in range(NKH)]
                    for ci in range(NCC):
                        for j0 in range(NKH):
                            nc.tensor.matmul(
                                ktp[j0][:],
                                wkq[:, ci * P: ci * P + 64],
                                xt[:, ci * T + j0 * NJ: ci * T + (j0 + 1) * NJ],
                                start=(ci == 0),
                                stop=(ci == NCC - 1),
                            )
                        for j0 in range(NKH):
                            nc.tensor.matmul(
                                qtp[j0][:],
                                wkq[:, ci * P + 64:(ci + 1) * P],
                                xt[:, ci * T + j0 * NJ: ci * T + (j0 + 1) * NJ],
                                start=(ci == 0),
                                stop=(ci == NCC - 1),
                            )
                    for j0 in range(NKH):
                        nc.vector.tensor_copy(
                            kt[:, j0 * NJ:(j0 + 1) * NJ], ktp[j0][:]
                        )
                        nc.vector.tensor_copy(
                            qt[:, j0 * NJ:(j0 + 1) * NJ], qtp[j0][:]
                        )
                    vtp = [psA.tile([64, NJ], F32, tag="pa", name=f"vtp{j}")
                           for j in range(NKH)]
                    kt2 = [psA.tile([64, NJ], F32, tag="pa", name=f"kt2{j}")
                           for j in range(NKH)]
                    for ci in range(NCC):
                        for j0 in range(NKH):
                            nc.tensor.matmul(
                                vtp[j0][:],
                                wv[:, ci * HS:(ci + 1) * HS],
                                xt[:, ci * T + j0 * NJ: ci * T + (j0 + 1) * NJ],
                                start=(ci == 0),
                                stop=(ci == NCC - 1),
                            )
                        for j0 in range(NKH):
                            nc.tensor.matmul(
                                kt2[j0][:],
                                wkq[:, ci * P: ci * P + 64],
                                xt[:, ci * T + TH + j0 * NJ:
                                   ci * T + TH + (j0 + 1) * NJ],
                                start=(ci == 0),
                                stop=(ci == NCC - 1),
                            )
                    for j0 in range(NKH):
                        nc.scalar.copy(
                            vtt[:, j0 * NJ:(j0 + 1) * NJ], vtp[j0][:]
                        )
                        nc.vector.tensor_copy(
                            kt[:, TH + j0 * NJ: TH + (j0 + 1) * NJ], kt2[j0][:]
                        )
                    # transposes ride the same rotating pool
                    for kg in range(4):
                        vtr = psA.tile([P, 4 * HS], F16, tag="pa", name=f"vtr{kg}")
                        for kk in range(4):
                            k = kg * 4 + kk
                            nc.tensor.transpose(
                                vtr[:, kk * HS:(kk + 1) * HS],
                                vtt[:, k * P:(k + 1) * P],
                                idh[:],
                            )
                        nc.vector.tensor_copy(
                            v[:, kg * 4 * HS:(kg + 1) * 4 * HS], vtr[:]
                        )

            # ---- phase B: scores, masked exp, row-sums, interleaved AV ----
            with (
                tc.tile_pool(name="psB", bufs=2, space="PSUM") as psB,
                tc.tile_pool(name="psC", bufs=1, space="PSUM") as psC,
                tc.tile_pool(name="pU", bufs=2) as pU,
                tc.tile_pool(name="pE", bufs=3) as pE,
                tc.tile_pool(name="pL", bufs=2) as pL,
            ):
                otp = psC.tile([P, TH], F32, tag="otp")  # packed AV accumulator

                def av_matmuls(k, u, jh):
                    # 4 of block k's AV matmuls: output partition half jh
                    for j2 in range(4):
                        nc.tensor.matmul(
                            otp[jh * 64:(jh + 1) * 64, j2 * NJ:(j2 + 1) * NJ],
                            vt[:, k * HS:(k + 1) * HS],
                            u[:, jh * TH + j2 * NJ: jh * TH + (j2 + 1) * NJ],
                            start=(k == 0),
                            stop=(k == NB - 1),
                            skip_group_check=True,
                        )

                def warm_fill(n):
                    # junk matmuls to keep the PE streaming where no AV work
                    # exists yet (start of phase B); output is overwritten by
                    # the next real start=True matmul on that buffer
                    wj = psB.tile([P, NQ], F32, tag="sp", name="wjunk")
                    for _ in range(n):
                        nc.tensor.matmul(
                            wj[:, 0:P], idn[:], idn[:], start=True, stop=True
                        )

                def quarter(k, jq, mk, jqh, u, ll):
                    sp = psB.tile([P, NQ], F32, tag="sp", name="sp")
                    inj = _inject(k, jq)
                    if inj:
                        for jb in range(2):
                            nc.tensor.matmul(
                                sp[:, jb * NJ:(jb + 1) * NJ],
                                idn[:],
                                mk[:, jqh * NQ + jb * NJ:
                                   jqh * NQ + (jb + 1) * NJ],
                                start=True,
                                stop=False,
                            )
                    for jb in range(2):
                        nc.tensor.matmul(
                            sp[:, jb * NJ:(jb + 1) * NJ],
                            qt[:, k * P:(k + 1) * P],
                            kt[:, jq * NQ + jb * NJ: jq * NQ + (jb + 1) * NJ],
                            start=(not inj),
                            stop=True,
                        )
                    if inj:
                        nc.scalar.activation(
                            u[:, jq * NQ:(jq + 1) * NQ], sp[:], AF.Exp,
                            bias=-30.0, scale=0.125,
                            accum_out=ll[:, jq:jq + 1],
                        )
                    else:
                        er = pE.tile([P, NQ], F16, tag="er", name="er")
                        nc.scalar.activation(er[:], sp[:], AF.Exp, scale=0.125)
                        nc.vector.scalar_tensor_tensor(
                            out=u[:, jq * NQ:(jq + 1) * NQ],
                            in0=er[:],
                            scalar=1.0,
                            in1=mk[:, jqh * NQ:(jqh + 1) * NQ],
                            op0=ALU.mult,
                            op1=ALU.mult,
                            accum_out=ll[:, jq:jq + 1],
                        )

                prev = None  # (k, u) whose AV matmuls are still pending
                for k in range(NB):
                    u = pU.tile([P, T], F16, tag="u")
                    ll = pL.tile([P, 4], F32, tag="ll")
                    if k < 2:
                        mk0, mk1 = mk_pre[2 * k], mk_pre[2 * k + 1]
                    else:
                        mk0 = pM.tile([P, 2 * NQ], F8, tag="mk", name="mk0")
                        nc.sync.dma_start(
                            mk0[:], mk_d[k * P:(k + 1) * P, 0:2 * NQ]
                        )
                        mk1 = pM.tile([P, 2 * NQ], F8, tag="mk", name="mk1")
                        nc.sync.dma_start(
                            mk1[:], mk_d[k * P:(k + 1) * P, 2 * NQ:4 * NQ]
                        )
                    quarter(k, 0, mk0, 0, u, ll)
                    quarter(k, 1, mk0, 1, u, ll)
                    # previous block's AV matmuls slot here: their inputs are
                    # long since ready, so they keep the PE streaming while
                    # exp works through this block's quarters
                    if prev is not None:
                        av_matmuls(prev[0], prev[1], 0)
                    else:
                        warm_fill(8)
                    quarter(k, 2, mk1, 0, u, ll)
                    if prev is not None:
                        av_matmuls(prev[0], prev[1], 1)
                    else:
                        warm_fill(8)
                    quarter(k, 3, mk1, 1, u, ll)
                    lt = pL.tile([P, 1], F32, tag="lt")
                    nc.vector.tensor_reduce(lt[:], ll[:], AX.X, ALU.add)
                    rl = pL.tile([P, 1], F32, tag="rl")
                    nc.vector.reciprocal(rl[:], lt[:])
                    nc.vector.tensor_scalar(
                        out=vt[:, k * HS:(k + 1) * HS],
                        in0=v[:, k * HS:(k + 1) * HS],
                        scalar1=rl[:],
                        scalar2=VSCALE,
                        op0=ALU.mult,
                        op1=ALU.mult,
                    )
                    prev = (k, u)
                av_matmuls(prev[0], prev[1], 0)
                av_matmuls(prev[0], prev[1], 1)

                with tc.tile_pool(name="phO", bufs=1) as pO:
                    ot_sb = pO.tile([P, TH], F32, tag="ot_sb")
                    for hh in range(2):
                        cs = slice(hh * (TH // 2), (hh + 1) * (TH // 2))
                        nc.vector.tensor_copy(ot_sb[:, cs], otp[:, cs])
                        nc.sync.dma_start(ot_d[:, cs], ot_sb[:, cs])
            pM_cm.__exit__(None, None, None)

    nc.compile()
    _CACHE["nc"] = nc
    return nc


def _prep_inputs(X, dag, Wk, Wq, Wv):
    X = np.asarray(X, dtype=np.float32)
    dag = np.asarray(dag)
    wkq = np.concatenate(
        [np.asarray(Wk, dtype=np.float16), np.asarray(Wq, dtype=np.float16)], axis=1
    )
    wv16 = np.asarray(Wv, dtype=np.float16)
    keep8 = (dag != 0).astype(NP_F8)  # [T, T] keep mask
    idn = np.zeros((P, P), dtype=NP_F8)
    np.fill_diagonal(idn, NP_F8(240.0))
    idh = np.eye(64, dtype=np.float16)
    in_maps = []
    for core in range(8):
        b, h = divmod(core, 2)
        xb = X[b].astype(np.float16)
        if h == 0:
            xt = np.ascontiguousarray(xb.T)
            mk = np.ascontiguousarray(keep8[0:TH])
        else:
            xt = np.ascontiguousarray(np.roll(xb.T, -TH, axis=1))
            mk = np.ascontiguousarray(np.roll(keep8[TH:2 * TH], -TH, axis=1))
        in_maps.append(
            {"xt": xt, "mk": mk, "wkq": wkq, "wv": wv16, "idn": idn, "idh": idh}
        )
    return in_maps


def kernel(X, dag, Wk, Wq, Wv, _trace=False):
    nc = _build()
    in_maps = _prep_inputs(X, dag, Wk, Wq, Wv)
    res = run_bass_kernel_spmd(nc, in_maps, list(range(8)), trace=_trace)
    out = np.empty((B, T, HS), dtype=np.float32)
    for b in range(B):
        acc = None
        for h in range(2):
            ot = res.results[2 * b + h]["ot"]  # [128, 2048] packed
            full = np.concatenate([ot[0:64, :], ot[64:128, :]], axis=1)  # [64, T]
            if h == 1:
                full = np.roll(full, TH, axis=1)  # undo key roll
            acc = full if acc is None else acc + full
        o = acc.T / np.float32(VSCALE)
        out[b] = o / (1.0 + np.exp(-o))  # swish
    if _trace:
        return out, res
    return out


# revision 18
# speedup vs baseline: 1.0505x; 1.0208x over previous
"""Trainium2 Bass kernel for nn_Head (sparse attention head), v2.

Computation (per batch b):
    K = X @ Wk; Q = X @ Wq; V = X @ Wv                       # [T, HS]
    S = Q K^T / sqrt(HS)                                     # [T, T]
    A = softmax_row(where(dag==0, -inf, S))                  # row-wise over keys
    out[j, h] = sum_i A[i, j] V[i, h]   (transposed AV)      # [T, HS]
    return swish(out)

Sharding over 8 NeuronCores: core = (b, h) with b = batch (4), h = query-row
half (2); each core handles TH=2048 query rows against all T=4096 keys.
The host rolls X^T columns (and mask key columns identically) so the shard's
query rows always sit in xt columns [0, TH) -- one program serves both h=0
and h=1 cores; the host unrolls the outputs.

v2 structure (vs v1 baseline):
  - mask shipped as fp8 keep-mask {0,1} (half the DMA bytes of f16)
  - 1/4 of score quarters get the mask applied ON THE TENSOR ENGINE: a
    diag(240) fp8 stationary matmul adds 240*keep into the score PSUM and
    the activation uses bias=-30 so exp(0.125*s + 30*keep - 30) kills
    masked entries (exp(<= -24) flushes to 0 in f16); activation accum_out
    then yields the masked row-sum for free on the Scalar engine.
  - remaining quarters keep v1's DVE scalar_tensor_tensor (exp * mask with
    accumulate) so Tensor/Vector/Scalar all carry ~equal load.
  - the transposed-AV matmul accumulates into a partition-packed PSUM tile
    [128, 2048] (output column halves stacked along partitions); block k's
    AV matmuls are interleaved into block k+1's score quarters so the PE
    never idles (keeps the HAM clock-gate at 2.4 GHz) and phase C vanishes.
  - matmuls are grouped by stationary operand to minimize LDWEIGHTS churn;
    phase A runs contraction-chunk-major with one weight load per chunk.
"""

import sys

for _p in ("/opt/trn_rl_repo",):
    if _p not in sys.path:
        sys.path.append(_p)

import numpy as np
import ml_dtypes

import concourse.bacc as bacc
import concourse.mybir as mybir
import concourse.tile as tile
from concourse.bass_utils import run_bass_kernel_spmd

B, T, D, HS = 4, 4096, 512, 64
TH = T // 2          # query rows per core
P = 128              # partitions
NB = TH // P         # 16 query blocks per core
NCC = D // P         # 4 contraction chunks over D
NJ = 512             # phase-A matmul free dim / PSUM bank width
NQ = 1024            # phase-B quarter width (keys)
VSCALE = 1024.0      # dynamic-range scale folded into V/l

F8 = mybir.dt.float8e4
F16 = mybir.dt.float16
F32 = mybir.dt.float32
AF = mybir.ActivationFunctionType
ALU = mybir.AluOpType
AX = mybir.AxisListType

NP_F8 = ml_dtypes.float8_e4m3


def _inject(k, jq):
    # score quarters whose mask goes in via tensor-engine injection
    # (the rest use the DVE scalar_tensor_tensor path)
    return jq in (0, 3)


_CACHE = {}


def _build():
    if "nc" in _CACHE:
        return _CACHE["nc"]

    nc = bacc.Bacc("TRN2", target_bir_lowering=False, debug=False)

    # register the -30.0 activation-bias constant (only 0.0/1.0 exist by default)
    _bias_t = nc.alloc_sbuf_tensor("const-float32--30.0", [128, 1], F32)
    nc.gpsimd.memset(_bias_t.ap(), -30.0)
    nc.const_aps.aps[(F32, -30.0)] = _bias_t.ap()
    nc.all_engine_barrier()

    xt_d = nc.dram_tensor("xt", [D, T], F16, kind="ExternalInput").ap()
    mk_d = nc.dram_tensor("mk", [TH, T], F8, kind="ExternalInput").ap()
    wkq_d = nc.dram_tensor("wkq", [D, P], F16, kind="ExternalInput").ap()
    wv_d = nc.dram_tensor("wv", [D, HS], F16, kind="ExternalInput").ap()
    idn_d = nc.dram_tensor("idn", [P, P], F8, kind="ExternalInput").ap()
    idh_d = nc.dram_tensor("idh", [64, 64], F16, kind="ExternalInput").ap()
    ot_d = nc.dram_tensor("ot", [P, TH], F32, kind="ExternalOutput").ap()

    with tile.TileContext(nc) as tc:
        with tc.tile_pool(name="persist", bufs=1) as pp:
            kt = pp.tile([64, T], F16, tag="kt")         # K^T
            qt = pp.tile([64, TH], F16, tag="qt")        # Q^T (shard rows)
            v = pp.tile([P, NB * HS], F16, tag="v")      # V rows (shard)
            vt = pp.tile([P, NB * HS], F16, tag="vt")    # V/l * VSCALE
            idn = pp.tile([P, P], F8, tag="idn")         # diag(240)
            idh = pp.tile([64, 64], F16, tag="idh")      # f16 identity
            nc.sync.dma_start(idn[:], idn_d[:, :])
            nc.sync.dma_start(idh[:], idh_d[:, :])

            # PE warmup: dummy matmuls keep the PE busy through the HAM
            # activity window so the clock un-throttles to 2.4 GHz while
            # the X^T DMA streams in; they depend only on the tiny idn DMA.
            with tc.tile_pool(name="psW", bufs=1, space="PSUM") as psW:
                wsp = psW.tile([P, P], F32, tag="wsp")
                for _ in range(56):
                    nc.tensor.matmul(wsp[:], idn[:], idn[:], start=True, stop=True)

            # mask pool opens early: the first blocks' masks load before
            # the big X^T transfers so phase B can start the moment the
            # projections finish
            pM_cm = tc.tile_pool(name="pM", bufs=4)
            pM = pM_cm.__enter__()
            mk_pre = []
            for k in range(2):
                for jh in range(2):
                    mkp = pM.tile([P, 2 * NQ], F8, tag="mk", name=f"mkp{k}{jh}")
                    nc.sync.dma_start(
                        mkp[:],
                        mk_d[k * P:(k + 1) * P, jh * 2 * NQ:(jh + 1) * 2 * NQ],
                    )
                    mk_pre.append(mkp)

            # ---- phase A: load X^T / weights, compute K^T, Q^T, V ----
            # All projection outputs are partition-PACKED [128, *] (two 64-row
            # halves stacked) so every PSUM bank carries real work and the
            # whole phase runs chunk-major, riding the X^T DMA ladder.
            with tc.tile_pool(name="phA", bufs=1) as pA:
                xt = pA.tile([P, NCC * T], F16, tag="xt")
                wkq = pA.tile([P, NCC * P], F16, tag="wkq")
                wv = pA.tile([P, NCC * HS], F16, tag="wv")
                vtt = pA.tile([64, TH], F16, tag="vtt")  # V^T (pre-transpose)
                for ci in range(NCC):
                    cs = slice(ci * P, (ci + 1) * P)
                    nc.sync.dma_start(wkq[:, ci * P:(ci + 1) * P], wkq_d[cs, :])
                    nc.sync.dma_start(wv[:, ci * HS:(ci + 1) * HS], wv_d[cs, :])
                # fine-grained chunk-major DMA: 64 transfers ladder across the
                # 16 queues so chunk ci is resident ~3us after chunk ci-1
                NDB = 1024
                for half in range(2):
                    for ci in range(NCC):
                        cs = slice(ci * P, (ci + 1) * P)
                        for j0 in range(half * TH, half * TH + TH, NDB):
                            nc.sync.dma_start(
                                xt[:, ci * T + j0: ci * T + j0 + NDB],
                                xt_d[cs, j0:j0 + NDB],
                            )

                with tc.tile_pool(name="psA", bufs=8, space="PSUM") as psA:
                    # rotating 8-bank pool: K^T tiles flow out as Q^T/V^T
                    # tiles flow in, so projections pipeline with the copies
                    NKH = TH // NJ
                    ktp = [psA.tile([64, NJ], F32, tag="pa", name=f"ktp{j}")
                           for j in range(NKH)]
                    qtp = [psA.tile([64, NJ], F32, tag="pa", name=f"qtp{j}")
                           for j in range(NKH)]
                    for ci in range(NCC):
                        for j0 in range(NKH):
                            nc.tensor.matmul(
                                ktp[j0][:],
                                wkq[:, ci * P: ci * P + 64],
                                xt[:, ci * T + j0 * NJ: ci * T + (j0 + 1) * NJ],
                                start=(ci == 0),
                                stop=(ci == NCC - 1),
                            )
                        for j0 in range(NKH):
                            nc.tensor.matmul(
                                qtp[j0][:],
                                wkq[:, ci * P + 64:(ci + 1) * P],
                                xt[:, ci * T + j0 * NJ: ci * T + (j0 + 1) * NJ],
                                start=(ci == 0),
                                stop=(ci == NCC - 1),
                            )
                    for j0 in range(NKH):
                        nc.vector.tensor_copy(
                            kt[:, j0 * NJ:(j0 + 1) * NJ], ktp[j0][:]
                        )
                        nc.vector.tensor_copy(
                            qt[:, j0 * NJ:(j0 + 1) * NJ], qtp[j0][:]
                        )
                    vtp = [psA.tile([64, NJ], F32, tag="pa", name=f"vtp{j}")
                           for j in range(NKH)]
                    kt2 = [psA.tile([64, NJ], F32, tag="pa", name=f"kt2{j}")
                           for j in range(NKH)]
                    for ci in range(NCC):
                        for j0 in range(NKH):
                            nc.tensor.matmul(
                                vtp[j0][:],
                                wv[:, ci * HS:(ci + 1) * HS],
                                xt[:, ci * T + j0 * NJ: ci * T + (j0 + 1) * NJ],
                                start=(ci == 0),
                                stop=(ci == NCC - 1),
                            )
                        for j0 in range(NKH):
                            nc.tensor.matmul(
                                kt2[j0][:],
                                wkq[:, ci * P: ci * P + 64],
                                xt[:, ci * T + TH + j0 * NJ:
                                   ci * T + TH + (j0 + 1) * NJ],
                                start=(ci == 0),
                                stop=(ci == NCC - 1),
                            )
                    for j0 in range(NKH):
                        nc.scalar.copy(
                            vtt[:, j0 * NJ:(j0 + 1) * NJ], vtp[j0][:]
                        )
                        nc.vector.tensor_copy(
                            kt[:, TH + j0 * NJ: TH + (j0 + 1) * NJ], kt2[j0][:]
                        )
                    # transposes ride the same rotating pool
                    for kg in range(4):
                        vtr = psA.tile([P, 4 * HS], F16, tag="pa", name=f"vtr{kg}")
                        for kk in range(4):
                            k = kg * 4 + kk
                            nc.tensor.transpose(
                                vtr[:, kk * HS:(kk + 1) * HS],
                                vtt[:, k * P:(k + 1) * P],
                                idh[:],
                            )
                        nc.vector.tensor_copy(
                            v[:, kg * 4 * HS:(kg + 1) * 4 * HS], vtr[:]
                        )

            # ---- phase B: scores, masked exp, row-sums, interleaved AV ----
            with (
                tc.tile_pool(name="psB", bufs=2, space="PSUM") as psB,
                tc.tile_pool(name="psC", bufs=1, space="PSUM") as psC,
                tc.tile_pool(name="pU", bufs=3) as pU,
                tc.tile_pool(name="pE", bufs=3) as pE,
                tc.tile_pool(name="pL", bufs=2) as pL,
            ):
                otp = psC.tile([P, TH], F32, tag="otp")  # packed AV accumulator

                def av_matmuls(k, u, jh):
                    # 4 of block k's AV matmuls: output partition half jh
                    for j2 in range(4):
                        nc.tensor.matmul(
                            otp[jh * 64:(jh + 1) * 64, j2 * NJ:(j2 + 1) * NJ],
                            vt[:, k * HS:(k + 1) * HS],
                            u[:, jh * TH + j2 * NJ: jh * TH + (j2 + 1) * NJ],
                            start=(k == 0),
                            stop=(k == NB - 1),
                            skip_group_check=True,
                        )

                def warm_fill(n):
                    # junk matmuls to keep the PE streaming where no AV work
                    # exists yet (start of phase B); output is overwritten by
                    # the next real start=True matmul on that buffer
                    wj = psB.tile([P, NQ], F32, tag="sp", name="wjunk")
                    for _ in range(n):
                        nc.tensor.matmul(
                            wj[:, 0:P], idn[:], idn[:], start=True, stop=True
                        )

                def quarter(k, jq, mk, jqh, u, ll):
                    sp = psB.tile([P, NQ], F32, tag="sp", name="sp")
                    inj = _inject(k, jq)
                    if inj:
                        for jb in range(2):
                            nc.tensor.matmul(
                                sp[:, jb * NJ:(jb + 1) * NJ],
                                idn[:],
                                mk[:, jqh * NQ + jb * NJ:
                                   jqh * NQ + (jb + 1) * NJ],
                                start=True,
                                stop=False,
                            )
                    for jb in range(2):
                        nc.tensor.matmul(
                            sp[:, jb * NJ:(jb + 1) * NJ],
                            qt[:, k * P:(k + 1) * P],
                            kt[:, jq * NQ + jb * NJ: jq * NQ + (jb + 1) * NJ],
                            start=(not inj),
                            stop=True,
                        )
                    if inj:
                        nc.scalar.activation(
                            u[:, jq * NQ:(jq + 1) * NQ], sp[:], AF.Exp,
                            bias=-30.0, scale=0.125,
                            accum_out=ll[:, jq:jq + 1],
                        )
                    else:
                        er = pE.tile([P, NQ], F16, tag="er", name="er")
                        nc.scalar.activation(er[:], sp[:], AF.Exp, scale=0.125)
                        nc.vector.scalar_tensor_tensor(
                            out=u[:, jq * NQ:(jq + 1) * NQ],
                            in0=er[:],
                            scalar=1.0,
                            in1=mk[:, jqh * NQ:(jqh + 1) * NQ],
                            op0=ALU.mult,
                            op1=ALU.mult,
                            accum_out=ll[:, jq:jq + 1],
                        )

                prev = None  # (k, u) whose AV matmuls are still pending
                for k in range(NB):
                    u = pU.tile([P, T], F16, tag="u")
                    ll = pL.tile([P, 4], F32, tag="ll")
                    if k < 2:
                        mk0, mk1 = mk_pre[2 * k], mk_pre[2 * k + 1]
                    else:
                        mk0 = pM.tile([P, 2 * NQ], F8, tag="mk", name="mk0")
                        nc.sync.dma_start(
                            mk0[:], mk_d[k * P:(k + 1) * P, 0:2 * NQ]
                        )
                        mk1 = pM.tile([P, 2 * NQ], F8, tag="mk", name="mk1")
                        nc.sync.dma_start(
                            mk1[:], mk_d[k * P:(k + 1) * P, 2 * NQ:4 * NQ]
                        )
                    quarter(k, 0, mk0, 0, u, ll)
                    quarter(k, 1, mk0, 1, u, ll)
                    # previous block's AV matmuls slot here: their inputs are
                    # long since ready, so they keep the PE streaming while
                    # exp works through this block's quarters
                    if prev is not None:
                        av_matmuls(prev[0], prev[1], 0)
                    else:
                        warm_fill(8)
                    quarter(k, 2, mk1, 0, u, ll)
                    if prev is not None:
                        av_matmuls(prev[0], prev[1], 1)
                    else:
                        warm_fill(8)
                    quarter(k, 3, mk1, 1, u, ll)
                    lt = pL.tile([P, 1], F32, tag="lt")
                    nc.vector.tensor_reduce(lt[:], ll[:], AX.X, ALU.add)
                    rl = pL.tile([P, 1], F32, tag="rl")
                    nc.vector.reciprocal(rl[:], lt[:])
                    nc.vector.tensor_scalar(
                        out=vt[:, k * HS:(k + 1) * HS],
                        in0=v[:, k * HS:(k + 1) * HS],
                        scalar1=rl[:],
                        scalar2=VSCALE,
                        op0=ALU.mult,
                        op1=ALU.mult,
                    )
                    prev = (k, u)
                av_matmuls(prev[0], prev[1], 0)
                av_matmuls(prev[0], prev[1], 1)

                with tc.tile_pool(name="phO", bufs=1) as pO:
                    ot_sb = pO.tile([P, TH], F32, tag="ot_sb")
                    nc.vector.tensor_copy(ot_sb[:], otp[:])
                    nc.sync.dma_start(ot_d[:, :], ot_sb[:])
            pM_cm.__exit__(None, None, None)

    nc.compile()
    _CACHE["nc"] = nc
    return nc


def _prep_inputs(X, dag, Wk, Wq, Wv):
    X = np.asarray(X, dtype=np.float32)
    dag = np.asarray(dag)
    wkq = np.concatenate(
        [np.asarray(Wk, dtype=np.float16), np.asarray(Wq, dtype=np.float16)], axis=1
    )
    wv16 = np.asarray(Wv, dtype=np.float16)
    keep8 = (dag != 0).astype(NP_F8)  # [T, T] keep mask
    idn = np.zeros((P, P), dtype=NP_F8)
    np.fill_diagonal(idn, NP_F8(240.0))
    idh = np.eye(64, dtype=np.float16)
    in_maps = []
    for core in range(8):
        b, h = divmod(core, 2)
        xb = X[b].astype(np.float16)
        if h == 0:
            xt = np.ascontiguousarray(xb.T)
            mk = np.ascontiguousarray(keep8[0:TH])
        else:
            xt = np.ascontiguousarray(np.roll(xb.T, -TH, axis=1))
            mk = np.ascontiguousarray(np.roll(keep8[TH:2 * TH], -TH, axis=1))
        in_maps.append(
            {"xt": xt, "mk": mk, "wkq": wkq, "wv": wv16, "idn": idn, "idh": idh}
        )
    return in_maps


def kernel(X, dag, Wk, Wq, Wv, _trace=False):
    nc = _build()
    in_maps = _prep_inputs(X, dag, Wk, Wq, Wv)
    res = run_bass_kernel_spmd(nc, in_maps, list(range(8)), trace=_trace)
    out = np.empty((B, T, HS), dtype=np.float32)
    for b in range(B):
        acc = None
        for h in range(2):
            ot = res.results[2 * b + h]["ot"]  # [128, 2048] packed
            full = np.concatenate([ot[0:64, :], ot[64:128, :]], axis=1)  # [64, T]
            if h == 1:
                full = np.roll(full, TH, axis=1)  # undo key roll
            acc = full if acc is None else acc + full
        o = acc.T / np.float32(VSCALE)
        out[b] = o / (1.0 + np.exp(-o))  # swish
    if _trace:
        return out, res
    return out


# revision 19
# speedup vs baseline: 1.0639x; 1.0127x over previous
"""Trainium2 Bass kernel for nn_Head (sparse attention head), v2.

Computation (per batch b):
    K = X @ Wk; Q = X @ Wq; V = X @ Wv                       # [T, HS]
    S = Q K^T / sqrt(HS)                                     # [T, T]
    A = softmax_row(where(dag==0, -inf, S))                  # row-wise over keys
    out[j, h] = sum_i A[i, j] V[i, h]   (transposed AV)      # [T, HS]
    return swish(out)

Sharding over 8 NeuronCores: core = (b, h) with b = batch (4), h = query-row
half (2); each core handles TH=2048 query rows against all T=4096 keys.
The host rolls X^T columns (and mask key columns identically) so the shard's
query rows always sit in xt columns [0, TH) -- one program serves both h=0
and h=1 cores; the host unrolls the outputs.

v2 structure (vs v1 baseline):
  - mask shipped as fp8 keep-mask {0,1} (half the DMA bytes of f16)
  - 1/4 of score quarters get the mask applied ON THE TENSOR ENGINE: a
    diag(240) fp8 stationary matmul adds 240*keep into the score PSUM and
    the activation uses bias=-30 so exp(0.125*s + 30*keep - 30) kills
    masked entries (exp(<= -24) flushes to 0 in f16); activation accum_out
    then yields the masked row-sum for free on the Scalar engine.
  - remaining quarters keep v1's DVE scalar_tensor_tensor (exp * mask with
    accumulate) so Tensor/Vector/Scalar all carry ~equal load.
  - the transposed-AV matmul accumulates into a partition-packed PSUM tile
    [128, 2048] (output column halves stacked along partitions); block k's
    AV matmuls are interleaved into block k+1's score quarters so the PE
    never idles (keeps the HAM clock-gate at 2.4 GHz) and phase C vanishes.
  - matmuls are grouped by stationary operand to minimize LDWEIGHTS churn;
    phase A runs contraction-chunk-major with one weight load per chunk.
"""

import sys

for _p in ("/opt/trn_rl_repo",):
    if _p not in sys.path:
        sys.path.append(_p)

import numpy as np
import ml_dtypes

import concourse.bacc as bacc
import concourse.mybir as mybir
import concourse.tile as tile
from concourse.bass_utils import run_bass_kernel_spmd

B, T, D, HS = 4, 4096, 512, 64
TH = T // 2          # query rows per core
P = 128              # partitions
NB = TH // P         # 16 query blocks per core
NCC = D // P         # 4 contraction chunks over D
NJ = 512             # phase-A matmul free dim / PSUM bank width
NQ = 1024            # phase-B quarter width (keys)
VSCALE = 1024.0      # dynamic-range scale folded into V/l

F8 = mybir.dt.float8e4
F16 = mybir.dt.float16
F32 = mybir.dt.float32
AF = mybir.ActivationFunctionType
ALU = mybir.AluOpType
AX = mybir.AxisListType

NP_F8 = ml_dtypes.float8_e4m3


def _inject(k, jq):
    # score quarters whose mask goes in via tensor-engine injection
    # (the rest use the DVE scalar_tensor_tensor path)
    return jq in (0, 3)


_CACHE = {}


def _build():
    if "nc" in _CACHE:
        return _CACHE["nc"]

    nc = bacc.Bacc("TRN2", target_bir_lowering=False, debug=False)

    # register the -30.0 activation-bias constant (only 0.0/1.0 exist by default)
    _bias_t = nc.alloc_sbuf_tensor("const-float32--30.0", [128, 1], F32)
    nc.gpsimd.memset(_bias_t.ap(), -30.0)
    nc.const_aps.aps[(F32, -30.0)] = _bias_t.ap()
    nc.all_engine_barrier()

    xt_d = nc.dram_tensor("xt", [D, T], F16, kind="ExternalInput").ap()
    mk_d = nc.dram_tensor("mk", [TH, T], F8, kind="ExternalInput").ap()
    wkq_d = nc.dram_tensor("wkq", [D, P], F16, kind="ExternalInput").ap()
    wv_d = nc.dram_tensor("wv", [D, HS], F16, kind="ExternalInput").ap()
    idn_d = nc.dram_tensor("idn", [P, P], F8, kind="ExternalInput").ap()
    idh_d = nc.dram_tensor("idh", [64, 64], F16, kind="ExternalInput").ap()
    ot_d = nc.dram_tensor("ot", [P, TH], F32, kind="ExternalOutput").ap()

    with tile.TileContext(nc) as tc:
        with tc.tile_pool(name="persist", bufs=1) as pp:
            kt = pp.tile([64, T], F16, tag="kt")         # K^T
            qt = pp.tile([64, TH], F16, tag="qt")        # Q^T (shard rows)
            v = pp.tile([P, NB * HS], F16, tag="v")      # V rows (shard)
            vt = pp.tile([P, NB * HS], F16, tag="vt")    # V/l * VSCALE
            idn = pp.tile([P, P], F8, tag="idn")         # diag(240)
            idh = pp.tile([64, 64], F16, tag="idh")      # f16 identity
            nc.sync.dma_start(idn[:], idn_d[:, :])
            nc.sync.dma_start(idh[:], idh_d[:, :])

            # PE warmup: dummy matmuls keep the PE busy through the HAM
            # activity window so the clock un-throttles to 2.4 GHz while
            # the X^T DMA streams in; they depend only on the tiny idn DMA.
            with tc.tile_pool(name="psW", bufs=1, space="PSUM") as psW:
                wsp = psW.tile([P, P], F32, tag="wsp")
                for _ in range(120):
                    nc.tensor.matmul(wsp[:], idn[:], idn[:], start=True, stop=True)

            # mask pool opens early: the first blocks' masks load before
            # the big X^T transfers so phase B can start the moment the
            # projections finish
            pM_cm = tc.tile_pool(name="pM", bufs=4)
            pM = pM_cm.__enter__()

            # ---- phase A: load X^T / weights, compute K^T, Q^T, V ----
            # All projection outputs are partition-PACKED [128, *] (two 64-row
            # halves stacked) so every PSUM bank carries real work and the
            # whole phase runs chunk-major, riding the X^T DMA ladder.
            with tc.tile_pool(name="phA", bufs=1) as pA:
                xt = pA.tile([P, NCC * T], F16, tag="xt")
                wkq = pA.tile([P, NCC * P], F16, tag="wkq")
                wv = pA.tile([P, NCC * HS], F16, tag="wv")
                vtt = pA.tile([64, TH], F16, tag="vtt")  # V^T (pre-transpose)
                for ci in range(NCC):
                    cs = slice(ci * P, (ci + 1) * P)
                    nc.sync.dma_start(wkq[:, ci * P:(ci + 1) * P], wkq_d[cs, :])
                    nc.sync.dma_start(wv[:, ci * HS:(ci + 1) * HS], wv_d[cs, :])
                # fine-grained chunk-major DMA: 64 transfers ladder across the
                # 16 queues so chunk ci is resident ~3us after chunk ci-1
                NDB = 1024
                for half in range(2):
                    for ci in range(NCC):
                        cs = slice(ci * P, (ci + 1) * P)
                        for j0 in range(half * TH, half * TH + TH, NDB):
                            nc.sync.dma_start(
                                xt[:, ci * T + j0: ci * T + j0 + NDB],
                                xt_d[cs, j0:j0 + NDB],
                            )

                # prefetch the first blocks' masks now that the X^T
                # transfers own the queue heads
                mk_pre = []
                for k in range(2):
                    for jh in range(2):
                        mkp = pM.tile([P, 2 * NQ], F8, tag="mk", name=f"mkp{k}{jh}")
                        nc.sync.dma_start(
                            mkp[:],
                            mk_d[k * P:(k + 1) * P,
                                 jh * 2 * NQ:(jh + 1) * 2 * NQ],
                        )
                        mk_pre.append(mkp)

                with tc.tile_pool(name="psA", bufs=8, space="PSUM") as psA:
                    # rotating 8-bank pool: K^T tiles flow out as Q^T/V^T
                    # tiles flow in, so projections pipeline with the copies
                    NKH = TH // NJ
                    ktp = [psA.tile([64, NJ], F32, tag="pa", name=f"ktp{j}")
                           for j in range(NKH)]
                    qtp = [psA.tile([64, NJ], F32, tag="pa", name=f"qtp{j}")
                           for j in range(NKH)]
                    for ci in range(NCC):
                        for j0 in range(NKH):
                            nc.tensor.matmul(
                                ktp[j0][:],
                                wkq[:, ci * P: ci * P + 64],
                                xt[:, ci * T + j0 * NJ: ci * T + (j0 + 1) * NJ],
                                start=(ci == 0),
                                stop=(ci == NCC - 1),
                            )
                        for j0 in range(NKH):
                            nc.tensor.matmul(
                                qtp[j0][:],
                                wkq[:, ci * P + 64:(ci + 1) * P],
                                xt[:, ci * T + j0 * NJ: ci * T + (j0 + 1) * NJ],
                                start=(ci == 0),
                                stop=(ci == NCC - 1),
                            )
                    for j0 in range(NKH):
                        nc.vector.tensor_copy(
                            kt[:, j0 * NJ:(j0 + 1) * NJ], ktp[j0][:]
                        )
                        nc.vector.tensor_copy(
                            qt[:, j0 * NJ:(j0 + 1) * NJ], qtp[j0][:]
                        )
                    vtp = [psA.tile([64, NJ], F32, tag="pa", name=f"vtp{j}")
                           for j in range(NKH)]
                    kt2 = [psA.tile([64, NJ], F32, tag="pa", name=f"kt2{j}")
                           for j in range(NKH)]
                    for ci in range(NCC):
                        for j0 in range(NKH):
                            nc.tensor.matmul(
                                vtp[j0][:],
                                wv[:, ci * HS:(ci + 1) * HS],
                                xt[:, ci * T + j0 * NJ: ci * T + (j0 + 1) * NJ],
                                start=(ci == 0),
                                stop=(ci == NCC - 1),
                            )
                        for j0 in range(NKH):
                            nc.tensor.matmul(
                                kt2[j0][:],
                                wkq[:, ci * P: ci * P + 64],
                                xt[:, ci * T + TH + j0 * NJ:
                                   ci * T + TH + (j0 + 1) * NJ],
                                start=(ci == 0),
                                stop=(ci == NCC - 1),
                            )
                    for j0 in range(NKH):
                        nc.scalar.copy(
                            vtt[:, j0 * NJ:(j0 + 1) * NJ], vtp[j0][:]
                        )
                        nc.vector.tensor_copy(
                            kt[:, TH + j0 * NJ: TH + (j0 + 1) * NJ], kt2[j0][:]
                        )
                    # transposes ride the same rotating pool
                    for kg in range(4):
                        vtr = psA.tile([P, 4 * HS], F16, tag="pa", name=f"vtr{kg}")
                        for kk in range(4):
                            k = kg * 4 + kk
                            nc.tensor.transpose(
                                vtr[:, kk * HS:(kk + 1) * HS],
                                vtt[:, k * P:(k + 1) * P],
                                idh[:],
                            )
                        nc.vector.tensor_copy(
                            v[:, kg * 4 * HS:(kg + 1) * 4 * HS], vtr[:]
                        )

            # ---- phase B: scores, masked exp, row-sums, interleaved AV ----
            with (
                tc.tile_pool(name="psB", bufs=2, space="PSUM") as psB,
                tc.tile_pool(name="psC", bufs=1, space="PSUM") as psC,
                tc.tile_pool(name="pU", bufs=3) as pU,
                tc.tile_pool(name="pE", bufs=3) as pE,
                tc.tile_pool(name="pL", bufs=2) as pL,
            ):
                otp = psC.tile([P, TH], F32, tag="otp")  # packed AV accumulator

                def av_matmuls(k, u, jh):
                    # 4 of block k's AV matmuls: output partition half jh
                    for j2 in range(4):
                        nc.tensor.matmul(
                            otp[jh * 64:(jh + 1) * 64, j2 * NJ:(j2 + 1) * NJ],
                            vt[:, k * HS:(k + 1) * HS],
                            u[:, jh * TH + j2 * NJ: jh * TH + (j2 + 1) * NJ],
                            start=(k == 0),
                            stop=(k == NB - 1),
                            skip_group_check=True,
                        )

                def warm_fill(n):
                    # junk matmuls to keep the PE streaming where no AV work
                    # exists yet (start of phase B); output is overwritten by
                    # the next real start=True matmul on that buffer
                    wj = psB.tile([P, NQ], F32, tag="sp", name="wjunk")
                    for _ in range(n):
                        nc.tensor.matmul(
                            wj[:, 0:P], idn[:], idn[:], start=True, stop=True
                        )

                def quarter(k, jq, mk, jqh, u, ll):
                    sp = psB.tile([P, NQ], F32, tag="sp", name="sp")
                    inj = _inject(k, jq)
                    if inj:
                        for jb in range(2):
                            nc.tensor.matmul(
                                sp[:, jb * NJ:(jb + 1) * NJ],
                                idn[:],
                                mk[:, jqh * NQ + jb * NJ:
                                   jqh * NQ + (jb + 1) * NJ],
                                start=True,
                                stop=False,
                            )
                    for jb in range(2):
                        nc.tensor.matmul(
                            sp[:, jb * NJ:(jb + 1) * NJ],
                            qt[:, k * P:(k + 1) * P],
                            kt[:, jq * NQ + jb * NJ: jq * NQ + (jb + 1) * NJ],
                            start=(not inj),
                            stop=True,
                        )
                    if inj:
                        nc.scalar.activation(
                            u[:, jq * NQ:(jq + 1) * NQ], sp[:], AF.Exp,
                            bias=-30.0, scale=0.125,
                            accum_out=ll[:, jq:jq + 1],
                        )
                    else:
                        er = pE.tile([P, NQ], F16, tag="er", name="er")
                        nc.scalar.activation(er[:], sp[:], AF.Exp, scale=0.125)
                        nc.vector.scalar_tensor_tensor(
                            out=u[:, jq * NQ:(jq + 1) * NQ],
                            in0=er[:],
                            scalar=1.0,
                            in1=mk[:, jqh * NQ:(jqh + 1) * NQ],
                            op0=ALU.mult,
                            op1=ALU.mult,
                            accum_out=ll[:, jq:jq + 1],
                        )

                prev = None  # (k, u) whose AV matmuls are still pending
                for k in range(NB):
                    u = pU.tile([P, T], F16, tag="u")
                    ll = pL.tile([P, 4], F32, tag="ll")
                    if k < 2:
                        mk0, mk1 = mk_pre[2 * k], mk_pre[2 * k + 1]
                    else:
                        mk0 = pM.tile([P, 2 * NQ], F8, tag="mk", name="mk0")
                        nc.sync.dma_start(
                            mk0[:], mk_d[k * P:(k + 1) * P, 0:2 * NQ]
                        )
                        mk1 = pM.tile([P, 2 * NQ], F8, tag="mk", name="mk1")
                        nc.sync.dma_start(
                            mk1[:], mk_d[k * P:(k + 1) * P, 2 * NQ:4 * NQ]
                        )
                    quarter(k, 0, mk0, 0, u, ll)
                    quarter(k, 1, mk0, 1, u, ll)
                    # previous block's AV matmuls slot here: their inputs are
                    # long since ready, so they keep the PE streaming while
                    # exp works through this block's quarters
                    if prev is not None:
                        av_matmuls(prev[0], prev[1], 0)
                    else:
                        warm_fill(8)
                    quarter(k, 2, mk1, 0, u, ll)
                    if prev is not None:
                        av_matmuls(prev[0], prev[1], 1)
                    else:
                        warm_fill(8)
                    quarter(k, 3, mk1, 1, u, ll)
                    lt = pL.tile([P, 1], F32, tag="lt")
                    nc.vector.tensor_reduce(lt[:], ll[:], AX.X, ALU.add)
                    rl = pL.tile([P, 1], F32, tag="rl")
                    nc.vector.reciprocal(rl[:], lt[:])
                    nc.vector.tensor_scalar(
                        out=vt[:, k * HS:(k + 1) * HS],
                        in0=v[:, k * HS:(k + 1) * HS],
                        scalar1=rl[:],
                        scalar2=VSCALE,
                        op0=ALU.mult,
                        op1=ALU.mult,
                    )
                    prev = (k, u)
                av_matmuls(prev[0], prev[1], 0)
                av_matmuls(prev[0], prev[1], 1)

                with tc.tile_pool(name="phO", bufs=1) as pO:
                    ot_sb = pO.tile([P, TH], F32, tag="ot_sb")
                    nc.vector.tensor_copy(ot_sb[:], otp[:])
                    nc.sync.dma_start(ot_d[:, :], ot_sb[:])
            pM_cm.__exit__(None, None, None)

    nc.compile()
    _CACHE["nc"] = nc
    return nc


def _prep_inputs(X, dag, Wk, Wq, Wv):
    X = np.asarray(X, dtype=np.float32)
    dag = np.asarray(dag)
    wkq = np.concatenate(
        [np.asarray(Wk, dtype=np.float16), np.asarray(Wq, dtype=np.float16)], axis=1
    )
    wv16 = np.asarray(Wv, dtype=np.float16)
    keep8 = (dag != 0).astype(NP_F8)  # [T, T] keep mask
    idn = np.zeros((P, P), dtype=NP_F8)
    np.fill_diagonal(idn, NP_F8(240.0))
    idh = np.eye(64, dtype=np.float16)
    in_maps = []
    for core in range(8):
        b, h = divmod(core, 2)
        xb = X[b].astype(np.float16)
        if h == 0:
            xt = np.ascontiguousarray(xb.T)
            mk = np.ascontiguousarray(keep8[0:TH])
        else:
            xt = np.ascontiguousarray(np.roll(xb.T, -TH, axis=1))
            mk = np.ascontiguousarray(np.roll(keep8[TH:2 * TH], -TH, axis=1))
        in_maps.append(
            {"xt": xt, "mk": mk, "wkq": wkq, "wv": wv16, "idn": idn, "idh": idh}
        )
    return in_maps


def kernel(X, dag, Wk, Wq, Wv, _trace=False):
    nc = _build()
    in_maps = _prep_inputs(X, dag, Wk, Wq, Wv)
    res = run_bass_kernel_spmd(nc, in_maps, list(range(8)), trace=_trace)
    out = np.empty((B, T, HS), dtype=np.float32)
    for b in range(B):
        acc = None
        for h in range(2):
            ot = res.results[2 * b + h]["ot"]  # [128, 2048] packed
            full = np.concatenate([ot[0:64, :], ot[64:128, :]], axis=1)  # [64, T]
            if h == 1:
                full = np.roll(full, TH, axis=1)  # undo key roll
            acc = full if acc is None else acc + full
        o = acc.T / np.float32(VSCALE)
        out[b] = o / (1.0 + np.exp(-o))  # swish
    if _trace:
        return out, res
    return out
